# revision 9
# baseline (speedup 1.0000x reference)
"""CapsNet forward on 8 trn2 NeuronCores (Bass/Tile), pure data parallel.

Network (per reference):
  conv1 9x9 s1 (1->256) -> primary-caps conv 9x9 s2 (256->256) -> squash(8-dim)
  -> digit caps (with ROUTING_ITERS=1 the routing weights are exactly
  softmax(0)=0.1 and the final b_log update is dead code, so the whole routing
  block reduces to s = squash(0.1 * einsum) = one matmul with 0.1 folded into
  W_dig) -> logits=||s||, argmax mask -> MLP 160->512->1024->784 reconstruction.

Sharding: batch 512 split 8x64; all weights replicated. No collectives needed.

Layouts on device (per core, B=64):
  x padded to [64, 792]; im2col X81[81, b*560] built by 9 wide-row DMAs/subblock
  conv1: psum[128,400] = W1[81,co128].T @ X81[:, b-slice AP(28,20)(1,20)]
  H1[ci_tile][128, b*400 + y'*20 + x'] for the full 64 batch
  pc conv: accumulate 81 offsets x 2 ci tiles into psum[co128, (b,y,x)] with
    strided rhs AP (b:400)(y:40)(x:2) offset ky*20+kx; fp32r (full rate, N>=256)
  squash: n2 = G8.T @ H2^2 (group-8 partition sums); f = sqrt(n2)/(1+n2);
    f broadcast to 128 partitions via G2.T @ f; u_in = H2 * f_b
  digit: psum[64,160] = sum_kt u_in-slice[128,64].T @ Wdig[kt][128,160]
    (k = pos*256 + co ordering, host-permuted)
  squash + argmax mask on [64,10,16] (||s|| monotonic in n2 -> is_ge mask)
  PE-transpose r0 -> [160,64]; MLP with out-channels on partitions so biases
  are per-partition ACT Identity/Relu/Sigmoid; recon returned as [784,64].T
"""

import sys
from contextlib import ExitStack

if "/opt/trn_rl_repo" not in sys.path:
    sys.path.insert(0, "/opt/trn_rl_repo")

import numpy as np

import concourse.bass as bass
from concourse import bacc
import concourse.mybir as mybir
import concourse.tile as tile
from concourse.bass import ts

F32 = mybir.dt.float32
F32R = mybir.dt.float32r

NCORES = 8
B = 512
BL = B // NCORES  # 64 per core

# conv1 geometry
O1 = 20  # conv1 output spatial
XP = 792  # padded x row length (needs 783 + 9 slack for wide-row im2col)
SUB = 8  # conv1 batch sub-block
# pc conv geometry
O2 = 6
POS = O2 * O2  # 36
# pc-conv batch blocks and psum chunks (chunk N must be >=256 and <=512)
PC_BLOCKS = [(0, 32), (32, 32)]
PC_CHUNKS = [(0, 11), (11, 11), (22, 10)]  # local-b start, count -> N=396,396,360

TRACE = False
LAST = {}


def _ap(t, offset, dims):
    """AP over tensor handle/AP `t` with explicit [step,count] free dims."""
    if isinstance(t, bass.AP):
        return bass.AP(tensor=t.tensor, offset=t.offset + offset, ap=list(dims))
    return bass.AP(tensor=t, offset=offset, ap=list(dims))


def _sb_ap(tile_ap, npart, offset, dims):
    """AP into an SBUF tile: partition dim [step,npart] + custom free dims."""
    return bass.AP(
        tensor=tile_ap.tensor,
        offset=tile_ap.offset + offset,
        ap=[[tile_ap.ap[0][0], npart]] + list(dims),
    )


def build(nc: bass.Bass):
    xp = nc.declare_dram_parameter("xp", [BL, XP], F32, isOutput=False)
    w1t = nc.declare_dram_parameter("w1t", [81, 256], F32, isOutput=False)
    pcwt = nc.declare_dram_parameter("pcwt", [81, 256, 256], F32, isOutput=False)
    wdig = nc.declare_dram_parameter("wdig", [9216, 160], F32, isOutput=False)
    d1t = nc.declare_dram_parameter("d1t", [160, 512], F32, isOutput=False)
    d2t = nc.declare_dram_parameter("d2t", [512, 1024], F32, isOutput=False)
    d3t = nc.declare_dram_parameter("d3t", [1024, 784], F32, isOutput=False)
    biasq = nc.declare_dram_parameter("biasq", [128, 23], F32, isOutput=False)
    g8 = nc.declare_dram_parameter("g8", [128, 16], F32, isOutput=False)
    g2 = nc.declare_dram_parameter("g2", [16, 128], F32, isOutput=False)
    ident = nc.declare_dram_parameter("ident", [64, 64], F32, isOutput=False)

    s_out = nc.declare_dram_parameter("s_out", [BL, 160], F32, isOutput=True)
    recon_t = nc.declare_dram_parameter("recon_t", [784, BL], F32, isOutput=True)
    dbg_x81 = nc.declare_dram_parameter("dbg_x81", [81, 560], F32, isOutput=True)
    dbg_h1 = nc.declare_dram_parameter("dbg_h1", [128, 400], F32, isOutput=True)
    dbg_u = nc.declare_dram_parameter("dbg_u", [128, 72], F32, isOutput=True)
    dbg_spre = nc.declare_dram_parameter("dbg_spre", [64, 160], F32, isOutput=True)
    dbg_r0t = nc.declare_dram_parameter("dbg_r0t", [128, 64], F32, isOutput=True)

    # bias pack columns
    BC1 = 0  # conv1 bias, 2 cols
    BPC = 2  # pc bias, 2 cols
    BD1 = 4  # d1 bias, 4 cols
    BD2 = 8  # d2 bias, 8 cols
    BD3 = 16  # d3 bias, 7 cols

    with tile.TileContext(nc) as tc, ExitStack() as octx:
        const = octx.enter_context(tc.tile_pool(name="const", bufs=1))
        persist = octx.enter_context(tc.tile_pool(name="persist", bufs=1))

        w1_sb = const.tile([81, 256], F32R)
        nc.sync.dma_start(out=w1_sb, in_=w1t[:, :].bitcast(F32R))
        bias_sb = const.tile([128, 23], F32)
        nc.sync.dma_start(out=bias_sb, in_=biasq[:, :])
        g8_sb = const.tile([128, 16], F32R)
        nc.sync.dma_start(out=g8_sb, in_=g8[:, :].bitcast(F32R))
        g2_sb = const.tile([16, 128], F32R)
        nc.sync.dma_start(out=g2_sb, in_=g2[:, :].bitcast(F32R))
        id_sb = const.tile([64, 64], F32)
        nc.sync.dma_start(out=id_sb, in_=ident[:, :])

        # u_in for the full local batch, per co-tile; f = b*36 + pos
        u_in = [
            persist.tile([128, BL * POS], F32, name=f"u_in{t}", tag=f"u_in{t}")
            for t in range(2)
        ]

        # -------- Phases A+B interleaved per 32-batch block --------
        with (
            tc.tile_pool(name="h1p", bufs=1) as h1p,
            tc.tile_pool(name="x81p", bufs=2) as x81p,
            tc.tile_pool(name="w2p", bufs=2) as w2p,
            tc.tile_pool(name="h2p", bufs=2) as h2p,
            tc.tile_pool(name="sqp", bufs=1) as sqp,
            tc.tile_pool(name="fp", bufs=1) as fp,
            tc.tile_pool(name="pspc", bufs=1, space="PSUM") as pspc,
            tc.tile_pool(name="psaux", bufs=2, space="PSUM") as psaux,
        ):
            for bg0, nb in PC_BLOCKS:
                h1 = [
                    h1p.tile([128, nb * 400], F32R, name=f"h1_{t}", tag=f"h1_{t}")
                    for t in range(2)
                ]
                # conv1 for this block
                for sub in range(nb // SUB):
                    b0 = sub * SUB  # block-local batch offset
                    x81 = x81p.tile([81, SUB * 560], F32R, tag="x81", name="x81")
                    for ky in range(9):
                        nc.sync.dma_start(
                            out=x81[ky * 9 : (ky + 1) * 9, :],
                            in_=_ap(
                                xp,
                                (bg0 + b0) * XP + ky * 28,
                                [[1, 9], [XP, SUB], [1, 560]],
                            ).bitcast(F32R),
                        )
                    if bg0 == 0 and sub == 0:
                        nc.sync.dma_start(
                            out=dbg_x81[:, :],
                            in_=x81[:, 0:560].bitcast(F32),
                        )
                    for bl in range(SUB):
                        for t in range(2):
                            pc1 = psaux.tile([128, 512], F32, tag="aux", name="pc1")
                            rhs = _sb_ap(x81, 81, bl * 560, [[28, 20], [1, 20]])
                            nc.tensor.matmul(
                                pc1[:, :400],
                                w1_sb[:, ts(t, 128)],
                                rhs.bitcast(F32R),
                                start=True,
                                stop=True,
                            )
                            # parity-split write: f = b*400 + par*200 + y'*10 + xh
                            nc.scalar.activation(
                                out=_sb_ap(
                                    h1[t],
                                    128,
                                    (b0 + bl) * 400,
                                    [[10, 20], [1, 10], [200, 2]],
                                ),
                                in_=_sb_ap(
                                    pc1, 128, 0, [[20, 20], [2, 10], [1, 2]]
                                ),
                                func=mybir.ActivationFunctionType.Identity,
                                bias=bias_sb[:, BC1 + t : BC1 + t + 1],
                            )
                if bg0 == 0:
                    nc.sync.dma_start(
                        out=dbg_h1[:, :], in_=h1[0][:, 0:400].bitcast(F32)
                    )
                # pc conv: accumulate 81 offsets x 2 ci tiles
                ps = {}
                for t in range(2):
                    for c, (cb0, nbc) in enumerate(PC_CHUNKS):
                        pst = pspc.tile(
                            [128, 512], F32, tag=f"pc{t}{c}", name=f"pc{t}{c}"
                        )
                        ps[(t, c)] = pst[:, : nbc * POS]
                for off in range(81):
                    ky, kx = off // 9, off % 9
                    w2t = w2p.tile([128, 2, 256], F32R, tag="w2", name="w2t")
                    nc.sync.dma_start(
                        out=w2t,
                        in_=_ap(
                            pcwt,
                            off * 256 * 256,
                            [[256, 128], [128 * 256, 2], [1, 256]],
                        ).bitcast(F32R),
                    )
                    for ci in range(2):
                        for t in range(2):
                            for c, (cb0, nbc) in enumerate(PC_CHUNKS):
                                rhs = _sb_ap(
                                    h1[ci],
                                    128,
                                    cb0 * 400
                                    + (kx & 1) * 200
                                    + ky * 10
                                    + (kx >> 1),
                                    [[400, nbc], [20, 6], [1, 6]],
                                )
                                nc.tensor.matmul(
                                    ps[(t, c)],
                                    w2t[:, ci, ts(t, 128)],
                                    rhs,
                                    start=(off == 0 and ci == 0),
                                    stop=(off == 80 and ci == 1),
                                )
                # squash each (t, chunk): u_in[t] = H2 * f_broadcast
                for t in range(2):
                    for c, (cb0, nbc) in enumerate(PC_CHUNKS):
                        ncn = nbc * POS
                        h2c = h2p.tile([128, 512], F32, tag="h2", name="h2c")
                        nc.scalar.activation(
                            out=h2c[:, :ncn],
                            in_=ps[(t, c)],
                            func=mybir.ActivationFunctionType.Identity,
                            bias=bias_sb[:, BPC + t : BPC + t + 1],
                        )
                        sqc = sqp.tile([128, 512], F32R, tag="sq", name="sqc")
                        nc.vector.tensor_mul(sqc[:, :ncn], h2c[:, :ncn], h2c[:, :ncn])
                        n2t = psaux.tile([128, 512], F32, tag="aux", name="n2t")
                        n2c = n2t[:16]
                        nc.tensor.matmul(
                            n2c[:, :ncn],
                            g8_sb,
                            sqc[:, :ncn],
                            start=True,
                            stop=True,
                        )
                        onep = fp.tile([16, 512], F32, tag="onep", name="onep")
                        nc.vector.tensor_scalar_add(onep[:, :ncn], n2c[:, :ncn], 1.0)
                        rcp = fp.tile([16, 512], F32, tag="rcp", name="rcp")
                        nc.vector.reciprocal(rcp[:, :ncn], onep[:, :ncn])
                        rtn = fp.tile([16, 512], F32, tag="rtn", name="rtn")
                        nc.scalar.sqrt(rtn[:, :ncn], n2c[:, :ncn])
                        fc = fp.tile([16, 512], F32R, tag="fc", name="fc")
                        nc.vector.tensor_mul(fc[:, :ncn], rcp[:, :ncn], rtn[:, :ncn])
                        fbt = psaux.tile([128, 512], F32, tag="aux", name="fbt")
                        fb = fbt
                        nc.tensor.matmul(
                            fb[:, :ncn],
                            g2_sb,
                            fc[:, :ncn],
                            start=True,
                            stop=True,
                        )
                        nc.vector.tensor_mul(
                            u_in[t][:, (bg0 + cb0) * POS : (bg0 + cb0 + nbc) * POS],
                            h2c[:, :ncn],
                            fb[:, :ncn],
                        )

        # ---------------- Phase C: digit caps + mask + MLP ----------------
        with (
            tc.tile_pool(name="wdg", bufs=1) as wdgp,
            tc.tile_pool(name="mlpw", bufs=1) as mlpw,
            tc.tile_pool(name="act", bufs=1) as act,
            tc.tile_pool(name="psd", bufs=2, space="PSUM") as psd,
            tc.tile_pool(name="psm", bufs=2, space="PSUM") as psm,
        ):
            nc.sync.dma_start(out=dbg_u[:, :], in_=u_in[0][:, 0:72])
            wdg_sb = wdgp.tile([128, 72, 160], F32)
            nc.sync.dma_start(
                out=wdg_sb, in_=_ap(wdig, 0, [[160, 128], [128 * 160, 72], [1, 160]])
            )
            s_ps = psd.tile([64, 512], F32, tag="spre", name="s_ps")
            for kt in range(72):
                pos, t = kt // 2, kt % 2
                lhs = _sb_ap(u_in[t], 128, pos, [[POS, BL]])
                nc.tensor.matmul(
                    s_ps[:, :160],
                    lhs,
                    wdg_sb[:, kt, :],
                    start=(kt == 0),
                    stop=(kt == 71),
                )
            spre_sb = act.tile([64, 160], F32)
            nc.vector.tensor_copy(spre_sb, s_ps[:, :160])
            nc.sync.dma_start(out=dbg_spre[:, :], in_=spre_sb)
            # squash on [64, 10, 16] + argmax mask
            sq2 = act.tile([64, 160], F32)
            nc.scalar.square(sq2, s_ps[:, :160])
            n2 = act.tile([64, 10], F32)
            nc.vector.reduce_sum(
                out=n2,
                in_=sq2.rearrange("b (j o) -> b j o", j=10),
                axis=mybir.AxisListType.X,
            )
            onep2 = act.tile([64, 10], F32)
            nc.vector.tensor_scalar_add(onep2, n2, 1.0)
            rcp2 = act.tile([64, 10], F32)
            nc.vector.reciprocal(rcp2, onep2)
            rtn2 = act.tile([64, 10], F32)
            nc.scalar.sqrt(rtn2, n2)
            fsq = act.tile([64, 10], F32)
            nc.vector.tensor_mul(fsq, rcp2, rtn2)
            mx = act.tile([64, 1], F32)
            nc.vector.reduce_max(out=mx, in_=n2, axis=mybir.AxisListType.X)
            mask = act.tile([64, 10], F32)
            nc.vector.tensor_tensor(
                mask, n2, mx.to_broadcast((64, 10)), mybir.AluOpType.is_ge
            )
            s_sb = act.tile([64, 160], F32)
            nc.vector.tensor_tensor(
                s_sb.rearrange("b (j o) -> b j o", j=10),
                s_ps[:, :160].rearrange("b (j o) -> b j o", j=10),
                fsq[:, :, None].to_broadcast((64, 10, 16)),
                mybir.AluOpType.mult,
            )
            nc.sync.dma_start(out=s_out[:, :], in_=s_sb)
            r0 = act.tile([64, 160], F32)
            nc.vector.tensor_tensor(
                r0.rearrange("b (j o) -> b j o", j=10),
                s_sb.rearrange("b (j o) -> b j o", j=10),
                mask[:, :, None].to_broadcast((64, 10, 16)),
                mybir.AluOpType.mult,
            )
            # transpose r0 -> [160, 64] (two PE transposes)
            tp0 = psm.tile([128, 64], F32, tag="tp", name="tp0")
            nc.tensor.transpose(tp0, r0[:, 0:128], id_sb)
            r0t0 = act.tile([128, 64], F32)
            nc.vector.tensor_copy(r0t0, tp0)
            tp1 = psm.tile([128, 64], F32, tag="tp", name="tp1")
            nc.tensor.transpose(tp1[:32, :], r0[:, 128:160], id_sb)
            r0t1 = act.tile([32, 64], F32)
            nc.vector.tensor_copy(r0t1, tp1[:32, :])
            nc.sync.dma_start(out=dbg_r0t[:, :], in_=r0t0)

            # MLP weights
            d1_sb = mlpw.tile([128, 2, 512], F32)
            nc.sync.dma_start(
                out=d1_sb[:, 0, :], in_=_ap(d1t, 0, [[512, 128], [1, 512]])
            )
            nc.sync.dma_start(
                out=d1_sb[:32, 1, :], in_=_ap(d1t, 128 * 512, [[512, 32], [1, 512]])
            )
            d2_sb = mlpw.tile([128, 4, 1024], F32)
            nc.sync.dma_start(
                out=d2_sb, in_=_ap(d2t, 0, [[1024, 128], [128 * 1024, 4], [1, 1024]])
            )
            d3_sb = mlpw.tile([128, 8, 784], F32)
            nc.sync.dma_start(
                out=d3_sb, in_=_ap(d3t, 0, [[784, 128], [128 * 784, 8], [1, 784]])
            )

            # layer 1: [160] -> [512], relu
            r1 = act.tile([128, 4, 64], F32)
            for mt in range(4):
                p1 = psm.tile([128, 64], F32, tag="mlp", name="p1")
                nc.tensor.matmul(
                    p1, d1_sb[:, 0, ts(mt, 128)], r0t0, start=True, stop=False
                )
                nc.tensor.matmul(
                    p1,
                    d1_sb[:32, 1, ts(mt, 128)],
                    r0t1,
                    start=False,
                    stop=True,
                )
                nc.scalar.activation(
                    out=r1[:, mt, :],
                    in_=p1,
                    func=mybir.ActivationFunctionType.Relu,
                    bias=bias_sb[:, BD1 + mt : BD1 + mt + 1],
                )
            # layer 2: [512] -> [1024], relu
            r2 = act.tile([128, 8, 64], F32)
            for mt in range(8):
                p2 = psm.tile([128, 64], F32, tag="mlp", name="p2")
                for kt in range(4):
                    nc.tensor.matmul(
                        p2,
                        d2_sb[:, kt, ts(mt, 128)],
                        r1[:, kt, :],
                        start=(kt == 0),
                        stop=(kt == 3),
                    )
                nc.scalar.activation(
                    out=r2[:, mt, :],
                    in_=p2,
                    func=mybir.ActivationFunctionType.Relu,
                    bias=bias_sb[:, BD2 + mt : BD2 + mt + 1],
                )
            # layer 3: [1024] -> [784], sigmoid
            for mt in range(7):
                msz = 128 if mt < 6 else 16
                p3 = psm.tile([128, 64], F32, tag="mlp", name="p3")
                for kt in range(8):
                    nc.tensor.matmul(
                        p3[:msz, :],
                        d3_sb[:, kt, mt * 128 : mt * 128 + msz],
                        r2[:, kt, :],
                        start=(kt == 0),
                        stop=(kt == 7),
                    )
                rec = act.tile([128, 64], F32, tag="rec", name="rec")
                nc.scalar.activation(
                    out=rec[:msz, :],
                    in_=p3[:msz, :],
                    func=mybir.ActivationFunctionType.Sigmoid,
                    bias=bias_sb[:msz, BD3 + mt : BD3 + mt + 1],
                )
                nc.sync.dma_start(
                    out=_ap(recon_t, mt * 128 * BL, [[BL, msz], [1, BL]]),
                    in_=rec[:msz, :],
                )

    return nc


def _host_prep(inputs):
    """Host-side weight reshapes/permutations (pure layout, no math beyond 0.1 fold)."""
    f = lambda k: np.ascontiguousarray(np.asarray(inputs[k], dtype=np.float32))
    x = f("x").reshape(B, 28 * 28)
    xp = np.zeros((B, XP), np.float32)
    xp[:, :784] = x

    w1t = np.ascontiguousarray(f("conv1_w").reshape(256, 81).T)  # [81, 256]
    pcwt = np.ascontiguousarray(
        f("pc_w").reshape(256, 256, 81).transpose(2, 1, 0)
    )  # [off, ci, co]
    w5 = f("W_dig").reshape(32, 36, 10, 16, 8)
    wdig = np.ascontiguousarray(
        (np.float32(0.1) * w5).transpose(1, 0, 4, 2, 3).reshape(9216, 160)
    )
    d1t = np.ascontiguousarray(f("d1_w").T)  # [160, 512]
    d2t = np.ascontiguousarray(f("d2_w").T)  # [512, 1024]
    d3t = np.ascontiguousarray(f("d3_w").T)  # [1024, 784]

    biasq = np.zeros((128, 23), np.float32)
    c1b, pcb = f("conv1_b"), f("pc_b")
    d1b, d2b, d3b = f("d1_b"), f("d2_b"), f("d3_b")
    for t in range(2):
        biasq[:, 0 + t] = c1b[t * 128 : (t + 1) * 128]
        biasq[:, 2 + t] = pcb[t * 128 : (t + 1) * 128]
    for mt in range(4):
        biasq[:, 4 + mt] = d1b[mt * 128 : (mt + 1) * 128]
    for mt in range(8):
        biasq[:, 8 + mt] = d2b[mt * 128 : (mt + 1) * 128]
    for mt in range(6):
        biasq[:, 16 + mt] = d3b[mt * 128 : (mt + 1) * 128]
    biasq[:16, 22] = d3b[768:784]

    g8 = np.zeros((128, 16), np.float32)
    g8[np.arange(128), np.arange(128) // 8] = 1.0
    g2 = np.ascontiguousarray(g8.T)
    ident = np.eye(64, dtype=np.float32)

    shared = dict(
        w1t=w1t, pcwt=pcwt, wdig=wdig, d1t=d1t, d2t=d2t, d3t=d3t,
        biasq=biasq, g8=g8, g2=g2, ident=ident,
    )
    return xp, shared


_NC_CACHE = []


def kernel(**inputs):
    from concourse.bass_utils import run_bass_kernel_spmd

    xp, shared = _host_prep(inputs)
    if not _NC_CACHE:
        nc = bacc.Bacc(None)
        build(nc)
        nc.finalize()
        _NC_CACHE.append(nc)
    nc = _NC_CACHE[0]

    in_maps = [
        dict(shared, xp=np.ascontiguousarray(xp[c * BL : (c + 1) * BL]))
        for c in range(NCORES)
    ]
    res = run_bass_kernel_spmd(
        nc, in_maps, list(range(NCORES)), trace=TRACE,
        **({"tmpdir": LAST["tmpdir"]} if TRACE and "tmpdir" in LAST else {}),
    )
    LAST["res"] = res

    s = np.concatenate([r["s_out"] for r in res.results], axis=0).reshape(B, 10, 16)
    recon = np.concatenate(
        [np.ascontiguousarray(r["recon_t"].T) for r in res.results], axis=0
    )
    return s, recon
